# revision 12
# baseline (speedup 1.0000x reference)
"""Distributed attention kernel for Trainium2 (8 NeuronCores).

Sharding: B*H = 2*16 = 32 (batch, head) pairs over 8 cores.
Core c handles batch b = c//4 and global heads 4*(c%4) .. 4*(c%4)+3
(i.e. output columns (c%4)*256 : (c%4+1)*256 of the 1024-wide output).

Per-core kernel (compute in bf16, f32 PSUM accumulation):
  - q/k/v arrive column-blocked from host ([c, m, p, 512] order) so the
    K/Q projections and first score tiles start ~9us in, not ~40us:
    the exp engines (ScalarE table exp + VectorE Schraudolph bit-trick
    exp) are the critical path and must never idle.
  - every elementwise op (exp, PSUM->SBUF casts, mask/output scales) is
    routed to ScalarE or VectorE by a greedy build-time cost balancer.
  - projections: QWT/KWT in [d, s] layout, VW in [s, d] layout with
    mask-zeroed rows; scores transposed S_T[k, q] with head pairs
    row-packed (64x128 PE tiling); PV col-tiled (two heads in PE column
    halves) accumulating O_T [128, 512] per chunk; sumexp via M=1
    matmuls 4-up col-tiled; epilogue transposes O_T back with PE,
    folds the sumexp merge into a PE transpose against `sel`, batched
    reciprocal + per-partition scale, DMA out (t-major, host
    reassembles).
"""

import numpy as np
from collections import deque

HEADS = 16
DK = 64
DM = 1024
B = 2
S = 2048
HL = 4           # heads per core
NCOL = HL * DK   # 256 projection cols per core
NM = DM // 128   # 8 m-chunks
NKC = S // 128   # 16 k-chunks
NQC = S // 512   # 4 q-chunks (column blocks)
NSUB = 512 // 128
NBLK = 2 * NKC   # 32 score tiles of [128,512] per (t, qc)

# Schraudolph exp in bf16 bit space: bits = x*SCH_A + SCH_B (then int16
# round); includes the 1/sqrt(dk)=0.125 score scale.
SCH_A = 1.4426950408889634 * 128.0 * 0.125
SCH_B = 127.0 * 128.0 - 7.0

_CACHE = {}


def _build():
    from contextlib import ExitStack
    import concourse.bass as bass  # noqa: F401
    import concourse.mybir as mybir
    import concourse.bacc as bacc
    import concourse.tile as tile
    from concourse.alu_op_type import AluOpType

    f32 = mybir.dt.float32
    bf16 = mybir.dt.bfloat16
    i16 = mybir.dt.int16
    Exp = mybir.ActivationFunctionType.Exp

    nc = bacc.Bacc("TRN2", target_bir_lowering=False, debug=False, num_devices=8)

    # column-blocked inputs, SBUF-layout-identical: [128, (c m s)] so each
    # c-block is one fully contiguous DMA with 8KB partition lines
    qT = nc.dram_tensor("qT", [128, NQC * NM * 512], bf16, kind="ExternalInput").ap()
    kT = nc.dram_tensor("kT", [128, NQC * NM * 512], bf16, kind="ExternalInput").ap()
    vT = nc.dram_tensor("vT", [128, NQC * NM * 512], bf16, kind="ExternalInput").ap()
    # weights host-swizzled to the SBUF layout [128, NM*NCOL]
    wq = nc.dram_tensor("wq", [128, NM * NCOL], bf16, kind="ExternalInput").ap()
    wk = nc.dram_tensor("wk", [128, NM * NCOL], bf16, kind="ExternalInput").ap()
    wv = nc.dram_tensor("wv", [128, NM * NCOL], bf16, kind="ExternalInput").ap()
    msk = nc.dram_tensor("msk", [128, NKC], f32, kind="ExternalInput").ap()
    ident = nc.dram_tensor("ident", [128, 128], f32, kind="ExternalInput").ap()
    sel = nc.dram_tensor("sel", [128, 2], f32, kind="ExternalInput").ap()
    # t-major output: rows [t*2048 + q], 128 cols (heads 2t, 2t+1)
    out = nc.dram_tensor("out", [2 * S, 128], f32, kind="ExternalOutput").ap()

    with tile.TileContext(nc) as tc, ExitStack() as ctx:
        const = ctx.enter_context(tc.tile_pool(name="const", bufs=1))
        bigp = ctx.enter_context(tc.tile_pool(name="bigp", bufs=5, space="PSUM"))
        ovt2 = ctx.enter_context(tc.tile_pool(name="ovt2", bufs=2, space="PSUM"))
        sep = ctx.enter_context(tc.tile_pool(name="sep", bufs=1, space="PSUM"))
        ep = ctx.enter_context(tc.tile_pool(name="ep", bufs=56))
        otsp = ctx.enter_context(tc.tile_pool(name="otsp", bufs=2))
        serowp = ctx.enter_context(tc.tile_pool(name="serowp", bufs=2))
        rcp = ctx.enter_context(tc.tile_pool(name="rcp", bufs=4))
        outp = ctx.enter_context(tc.tile_pool(name="outp", bufs=8))

        # ---- persistent SBUF tensors ----
        xq = const.tile([128, NM * S], bf16, tag="xq")
        xk = const.tile([128, NM * S], bf16, tag="xk")
        xv = const.tile([128, NM * S], bf16, tag="xv")
        wq_sb = const.tile([128, NM * NCOL], bf16, tag="wq")
        wk_sb = const.tile([128, NM * NCOL], bf16, tag="wk")
        wv_sb = const.tile([128, NM * NCOL], bf16, tag="wv")
        m_sb = const.tile([128, NKC], f32, tag="m")
        mskb = const.tile([128, NKC], bf16, tag="mb")
        id_sb = const.tile([128, 128], f32, tag="id")
        id_b = const.tile([128, 128], bf16, tag="idb")
        sel_sb = const.tile([128, 2], f32, tag="sel")
        sel_b = const.tile([128, 2], bf16, tag="selb")
        qwt = const.tile([128, 2 * S], bf16, tag="qwt")    # [d(2 heads), s] x2
        kwt = const.tile([128, 2 * S], bf16, tag="kwt")
        vw = const.tile([128, NKC * HL * DK], bf16, tag="vw")
        vw_4d = vw[:, :].rearrange("p (k h c) -> p k h c", k=NKC, h=HL)

        # persistent PSUM sumexp accumulator (rows 0/32/64/96)
        ses = sep.tile([128, 512], f32, tag="ses", name="ses")

        # ---- greedy two-engine router for elementwise work ----
        # calibrated from baseline trace: ACT [128,512] psum->sbuf = 733ns,
        # DVE = 751ns
        busy = {"S": 0.0, "V": 0.0}

        def pick(n):
            c_s = n * 0.833 + 307.0
            c_v = n * 1.042 + 218.0
            if busy["V"] + c_v <= busy["S"] + c_s:
                busy["V"] += c_v
                return "V"
            busy["S"] += c_s
            return "S"

        def e_copy(dst, src, n):
            if pick(n) == "V":
                nc.vector.tensor_copy(dst, src)
            else:
                nc.scalar.copy(dst, src)

        def e_mul(dst, src, col, n):
            if pick(n) == "V":
                nc.vector.tensor_scalar_mul(dst, src, col)
            else:
                nc.scalar.mul(dst, src, col)

        def e_exp(et, st):
            if pick(512) == "V":
                nc.vector.tensor_scalar(
                    out=et[:, :].bitcast(i16),
                    in0=st[:, :],
                    scalar1=SCH_A,
                    scalar2=SCH_B,
                    op0=AluOpType.mult,
                    op1=AluOpType.add,
                )
            else:
                nc.scalar.activation(et[:, :], st[:, :], Exp, scale=0.125)

        # x SBUF column layout is c-major: col of (m, c, j) = (c*NM+m)*512+j
        def xs(x_sb, m, c, off=0, w=512):
            base = (c * NM + m) * 512 + off
            return x_sb[:, base: base + w]

        # ---- input DMA, column-blocked, in consumption order ----
        def dma_x(x_sb, src, c):
            nc.sync.dma_start(
                out=x_sb[:, c * NM * 512: (c + 1) * NM * 512],
                in_=src[:, c * NM * 512: (c + 1) * NM * 512],
            )

        nc.sync.dma_start(out=wq_sb[:, :], in_=wq)
        dma_x(xq, qT, 0)
        nc.sync.dma_start(out=wk_sb[:, :], in_=wk)
        dma_x(xk, kT, 0)
        nc.sync.dma_start(out=m_sb[:, :], in_=msk)
        nc.sync.dma_start(out=id_sb[:, :], in_=ident)
        nc.sync.dma_start(out=sel_sb[:, :], in_=sel)
        for c in range(1, NQC):
            dma_x(xk, kT, c)
        dma_x(xq, qT, 1)
        nc.sync.dma_start(out=wv_sb[:, :], in_=wv)
        dma_x(xq, qT, 2)
        for c in range(NQC):
            dma_x(xv, vT, c)
        dma_x(xq, qT, 3)

        # HAM pre-warm: the PE clock sits at 1.2 GHz until ~3.4us of
        # sustained activity; burn dummy matmuls while the first input DMAs
        # are in flight so the unthrottle happens before the projections
        warm = const.tile([128, 128], bf16, tag="warm")
        nc.vector.memset(warm[:, :], 0.0)
        warm_ps = bigp.tile([128, 128], f32, tag="big", name="warm_ps")
        for w in range(16):
            nc.tensor.matmul(
                warm_ps[:, :], lhsT=warm[:, :], rhs=warm[:, :],
                start=True, stop=True,
            )

        # one-time derived constants + sumexp-bank scrub (rows between the
        # 4 accumulator rows are read by the epilogue copy and must not be
        # NaN; matmul start=True only clears rows it writes)
        nc.vector.memset(ses[:, :], 0.0)
        nc.vector.tensor_copy(mskb[:, :], m_sb[:, :])
        nc.vector.tensor_copy(id_b[:, :], id_sb[:, :])
        nc.vector.tensor_copy(sel_b[:, :], sel_sb[:, :])
        busy["V"] += 3 * 300.0

        def proj_qk(w_sb, x_sb, dst, t, c):
            ps = bigp.tile([128, 512], f32, tag="big", name=f"pqk{t}_{c}")
            for m in range(NM):
                nc.tensor.matmul(
                    ps[:, :],
                    lhsT=w_sb[:, m * NCOL + t * 128: m * NCOL + t * 128 + 128],
                    rhs=xs(x_sb, m, c),
                    start=(m == 0),
                    stop=(m == NM - 1),
                )
            e_copy(dst[:, t * S + c * 512: t * S + c * 512 + 512], ps[:, :], 512)

        def proj_v(kb):
            ps = bigp.tile([128, NCOL], f32, tag="big", name=f"pv{kb}")
            for m in range(NM):
                nc.tensor.matmul(
                    ps[:, :],
                    lhsT=xs(xv, m, kb // 4, (kb % 4) * 128, 128),
                    rhs=wv_sb[:, m * NCOL: (m + 1) * NCOL],
                    start=(m == 0),
                    stop=(m == NM - 1),
                )
            e_mul(
                vw[:, kb * NCOL: (kb + 1) * NCOL],
                ps[:, :],
                m_sb[:, kb: kb + 1],
                NCOL,
            )

        class Chunk:
            """Incremental emitter for one (t, qc) attention chunk."""

            def __init__(self, ci, t, qc):
                self.ci, self.t, self.qc = ci, t, qc
                self.s_tiles = [None] * NBLK
                self.e_tiles = [None] * NBLK
                self.ov = None
                self.si = 0
                self.pi = 0
                self.zi = 0

            def emit_s(self, n):
                t, qc = self.t, self.qc
                todo = list(range(self.si, min(self.si + n, NBLK)))
                if not todo:
                    return
                self.si = todo[-1] + 1
                for blk in todo:
                    self.s_tiles[blk] = bigp.tile(
                        [128, 512], f32, tag="big", name=f"sps{t}_{qc}_{blk}"
                    )
                for blk in todo:
                    kc, a = divmod(blk, 2)
                    nc.tensor.matmul(
                        self.s_tiles[blk][:, :],
                        lhsT=kwt[
                            64 * a: 64 * a + 64,
                            t * S + kc * 128: t * S + kc * 128 + 128,
                        ],
                        rhs=qwt[
                            64 * a: 64 * a + 64,
                            t * S + qc * 512: t * S + qc * 512 + 512,
                        ],
                        start=True,
                        stop=True,
                        tile_position=(64 * a, 0),
                    )
                for blk in todo:
                    et = ep.tile(
                        [128, 512], bf16, tag="e", name=f"et{t}_{qc}_{blk}"
                    )
                    self.e_tiles[blk] = et
                    e_exp(et, self.s_tiles[blk])

            def emit_pv(self, n):
                t = self.t
                if self.ov is None:
                    self.ov = ovt2.tile(
                        [128, 512], f32, tag="ov", name=f"ov{t}_{self.qc}"
                    )
                blks = list(range(self.pi, min(self.pi + n, NBLK)))
                if not blks:
                    return
                self.pi = blks[-1] + 1
                for blk in blks:
                    kc, a = divmod(blk, 2)
                    # two interleaved per-head accumulation chains share this
                    # bank on disjoint partition halves; per-element
                    # has_written handles it, but the sim's coarse group
                    # checker must be bypassed
                    nc.tensor.matmul(
                        self.ov[64 * a: 64 * a + 64, :],
                        lhsT=vw_4d[:, kc, 2 * t + a, :],
                        rhs=self.e_tiles[blk][:, :],
                        start=(kc == 0),
                        stop=(kc == NKC - 1),
                        skip_group_check=True,
                        tile_position=(0, 64 * a),
                    )

            def emit_se(self, n):
                # sumexp slot p covers kcs (2p, 2p+1) x 2 heads, 4-up
                # col-tiled at positions 0/32/64/96
                slots = list(range(self.zi, min(self.zi + n, NKC // 2)))
                if not slots:
                    return
                self.zi = slots[-1] + 1
                for p in slots:
                    for j, (kc, a) in enumerate(
                        ((2 * p, 0), (2 * p, 1), (2 * p + 1, 0), (2 * p + 1, 1))
                    ):
                        nc.tensor.matmul(
                            ses[32 * j: 32 * j + 1, :],
                            lhsT=mskb[:, kc: kc + 1],
                            rhs=self.e_tiles[2 * kc + a][:, :],
                            start=(p == 0),
                            stop=(p == NKC // 2 - 1),
                            tile_position=(0, 32 * j),
                        )

            def done(self):
                return self.pi >= NBLK and self.zi >= NKC // 2

        def pre_epilogue(ch):
            # O_T to SBUF (PE cannot read PSUM) + transposes; independent of
            # the sumexp accumulator, so it can overlap the final se batch
            t, qc = ch.t, ch.qc
            ovs = otsp.tile([128, 512], bf16, tag="ots", name=f"ovs{t}_{qc}")
            e_copy(ovs[:, :], ch.ov[:, :], 512)
            ch.ovs = ovs
            t2 = ovt2.tile([128, 1024], bf16, tag="ov", name=f"t2{t}_{qc}")
            ch.t2 = t2
            t2_3d = t2[:, 0:512].rearrange("p (s c) -> p s c", s=NSUB)
            for sub in range(NSUB):
                nc.tensor.matmul(
                    t2_3d[:, sub, :],
                    lhsT=ovs[:, sub * 128: (sub + 1) * 128],
                    rhs=id_b[:, :],
                    is_transpose=True,
                    skip_group_check=True,
                )

        def epilogue(ch):
            t, qc = ch.t, ch.qc
            if getattr(ch, "ovs", None) is None:
                pre_epilogue(ch)
            serow = serowp.tile([97, 512], bf16, tag="ser", name=f"ser{t}_{qc}")
            e_copy(serow[:, :], ses[0:97, :], 512)
            t2 = ch.t2
            t2_3d = t2[:, 0:512].rearrange("p (s c) -> p s c", s=NSUB)
            t2se = t2[:, 512:528].bitcast(f32).rearrange(
                "p (s c) -> p s c", s=NSUB
            )
            for sub in range(NSUB):
                # sumexp gather+merge: [97,128]^T @ sel -> [128 q, 2 heads];
                # sel sums the kc-parity accumulator pair per head
                nc.tensor.matmul(
                    t2se[:, sub, :],
                    lhsT=serow[:, sub * 128: (sub + 1) * 128],
                    rhs=sel_b[0:97, :],
                    start=True,
                    stop=True,
                    skip_group_check=True,
                )
            rc = rcp.tile([128, NSUB * 2], f32, tag="rc", name=f"rc{t}_{qc}")
            rc3 = rc[:, :].rearrange("p (s c) -> p s c", s=NSUB)
            # batched reciprocal over all 4 subs x 2 heads in one op
            nc.vector.reciprocal_approx_fast(out=rc3[:, :, :], in_=t2se[:, :, :])
            busy["V"] += 250.0
            for sub in range(NSUB):
                o_out = outp.tile([128, 128], f32, tag="out", name=f"oo{t}_{qc}_{sub}")
                for a in range(2):
                    e_mul(
                        o_out[:, a * 64: (a + 1) * 64],
                        t2_3d[:, sub, a * 64: (a + 1) * 64],
                        rc[:, 2 * sub + a: 2 * sub + a + 1],
                        64,
                    )
                nc.sync.dma_start(
                    out=out[
                        t * S + qc * 512 + sub * 128:
                        t * S + qc * 512 + sub * 128 + 128, :
                    ],
                    in_=o_out[:, :],
                )

        # ---- schedule ----
        chunks = [Chunk(ci, ci // 4, ci % 4) for ci in range(8)]

        # phase A: chunk0 scores woven with t0 projections (exp starts ~9us)
        proj_qk(wq_sb, xq, qwt, 0, 0)
        proj_qk(wk_sb, xk, kwt, 0, 0)
        chunks[0].emit_s(8)
        for c in range(1, NQC):
            proj_qk(wk_sb, xk, kwt, 0, c)
            chunks[0].emit_s(8)

        # phase B: chunk1 scores + t1 kwt + first vprojs (xv starts landing)
        proj_qk(wq_sb, xq, qwt, 0, 1)
        b_fill = [
            lambda: proj_qk(wk_sb, xk, kwt, 1, 0),
            lambda: proj_qk(wk_sb, xk, kwt, 1, 1),
            lambda: proj_qk(wk_sb, xk, kwt, 1, 2),
            lambda: proj_qk(wk_sb, xk, kwt, 1, 3),
            lambda: (proj_v(0), proj_v(1)),
            lambda: proj_qk(wq_sb, xq, qwt, 0, 2),
            lambda: (proj_v(2), proj_v(3)),
            lambda: (proj_v(4), proj_v(5)),
        ]
        for i in range(8):
            chunks[1].emit_s(4)
            b_fill[i]()

        # phase B2: chunk2 scores + chunk0 pv/se + remaining vprojs
        b2_fill = [
            lambda: (proj_v(6), proj_v(7)),
            lambda: (proj_v(8), proj_v(9)),
            lambda: proj_qk(wq_sb, xq, qwt, 0, 3),
            lambda: (proj_v(10), proj_v(11)),
            lambda: (proj_v(12), proj_v(13)),
            lambda: (proj_v(14), proj_v(15)),
            lambda: proj_qk(wq_sb, xq, qwt, 1, 0),
            lambda: None,
        ]
        for i in range(8):
            chunks[2].emit_s(4)
            b2_fill[i]()
            if i >= 2:
                chunks[0].emit_pv(4)
            if i >= 4:
                chunks[0].emit_se(1)
        chunks[0].emit_pv(NBLK)
        chunks[0].emit_se(NKC // 2)
        epilogue(chunks[0])

        # phase C: steady state. chunk k scores while draining a global
        # pv/se backlog (chunks 1.. lag behind because vw lands late); a
        # 6-blocks/round pv quota converges the lag by ~chunk 5.
        pvq = deque(chunks[1:])
        epi_done = 1  # epilogue emitted for chunks < epi_done

        def drain(pv_budget):
            nonlocal epi_done
            while pv_budget > 0 and pvq:
                ch = pvq[0]
                # pv/se can only consume e-tiles whose scores are emitted;
                # se for chunk ci is safe once epilogue(ci-1) was emitted
                # (pvq pops in order, so that's structural)
                take = min(pv_budget, ch.si - ch.pi)
                if take > 0:
                    ch.emit_pv(take)
                    pv_budget -= take
                if ch.zi < NKC // 2 and ch.si >= 4 * (ch.zi + 1):
                    ch.emit_se(1)
                if ch.pi >= NBLK:
                    ch.emit_se(NKC // 2)
                    epilogue(ch)
                    epi_done = ch.ci + 1
                    pvq.popleft()
                    continue
                break

        late_q = {
            3: lambda: proj_qk(wq_sb, xq, qwt, 1, 1),
            4: lambda: proj_qk(wq_sb, xq, qwt, 1, 2),
            5: lambda: proj_qk(wq_sb, xq, qwt, 1, 3),
        }
        for k in range(3, 8):
            for i in range(8):
                chunks[k].emit_s(4)
                if i == 0 and k in late_q:
                    late_q[k]()
                drain(8)
        # tail: everything left (chunk 6/7 pv + se + epilogues)
        while pvq:
            drain(NBLK)

    nc.compile()
    return nc


def _get_nc():
    if "nc" not in _CACHE:
        _CACHE["nc"] = _build()
    return _CACHE["nc"]


def _shard_inputs(q, k, v, mask, Wq, Wk, Wv):
    import ml_dtypes

    bf16 = ml_dtypes.bfloat16
    q = np.asarray(q, np.float32)
    k = np.asarray(k, np.float32)
    v = np.asarray(v, np.float32)
    mask = np.asarray(mask, np.float32)
    Wq = np.asarray(Wq, np.float32)
    Wk = np.asarray(Wk, np.float32)
    Wv = np.asarray(Wv, np.float32)

    def _swz(w):
        # [1024, 256] -> SBUF layout [128, 8*256] (row p = concat_m W[m*128+p])
        return np.ascontiguousarray(
            w.reshape(NM, 128, NCOL).transpose(1, 0, 2).reshape(128, NM * NCOL)
        ).astype(bf16)

    def _blk(xT):
        # [1024, 2048] -> SBUF-identical c-major layout [128, (c m s)]
        return np.ascontiguousarray(
            xT.reshape(NM, 128, NQC, 512)
            .transpose(1, 2, 0, 3)
            .reshape(128, NQC * NM * 512)
        ).astype(bf16)

    ident = np.eye(128, dtype=np.float32)
    sel = np.zeros((128, 2), np.float32)
    sel[0, 0] = sel[64, 0] = 1.0
    sel[32, 1] = sel[96, 1] = 1.0
    qTs = [_blk(q[b].T) for b in range(B)]
    kTs = [_blk(k[b].T) for b in range(B)]
    vTs = [_blk(v[b].T) for b in range(B)]
    msks = [
        np.ascontiguousarray(mask[b].reshape(NKC, 128).T).astype(np.float32)
        for b in range(B)
    ]
    in_maps = []
    for c in range(8):
        b, j = c // 4, c % 4
        sl = slice(j * NCOL, (j + 1) * NCOL)
        in_maps.append(
            {
                "qT": qTs[b],
                "kT": kTs[b],
                "vT": vTs[b],
                "wq": _swz(Wq[:, sl]),
                "wk": _swz(Wk[:, sl]),
                "wv": _swz(Wv[:, sl]),
                "msk": msks[b],
                "ident": ident,
                "sel": sel,
            }
        )
    return in_maps


def _assemble(results):
    """results: list of 8 dicts with 'out' [2*S, 128] -> full [B, S, 1024]."""
    outp = np.empty((B, S, HEADS * DK), np.float32)
    for c in range(8):
        b, j = c // 4, c % 4
        o = np.asarray(results[c]["out"]).reshape(2, S, 128)
        outp[b, :, j * NCOL: j * NCOL + 128] = o[0]
        outp[b, :, j * NCOL + 128: j * NCOL + 256] = o[1]
    return outp


def kernel(q, k, v, mask, Wq, Wk, Wv):
    from concourse.bass_utils import run_bass_kernel_spmd

    nc = _get_nc()
    in_maps = _shard_inputs(q, k, v, mask, Wq, Wk, Wv)
    res = run_bass_kernel_spmd(nc, in_maps, core_ids=list(range(8))).results
    return _assemble(res)
